# revision 6
# baseline (speedup 1.0000x reference)
"""Trainium2 Bass kernel for nn_CoreDiffusion (GNN message passing + GRU + LayerNorm).

Algorithm (matches reference):
    for k in [K-1 .. 0]:
        res = relu(segment_sum(vals[k] * x[cols[k]], rows[k]))      # adj @ x
        h   = GRUCell(res, h)
    out = LayerNorm(h) * ln_g + ln_b

Distribution: destination-node sharding across 8 NeuronCores. The host
partitions edges by dest-row owner, groups them by (512-row dest group,
source half), sorts each group by local dest row and chunks into 128-edge
chunks. Because chunks are dest-sorted, each chunk's dests fit a 64-wide
window at a per-chunk static offset shared across cores (min-start schedule),
so the scatter matrix W is only [128, 64] per chunk (4x cheaper on DVE than
a full 256-wide supertile) and PSUM accumulates a [128, 512] group tile.

Per-core device pipeline, in feature-transposed space (partition = feature):
  - dma_gather pulls bf16 x[source] rows (256B each) from HBM into SBUF
    chunks of [128 edges, 128 feat].
  - One fused DVE tensor_scalar(is_equal, mult) per chunk builds the bf16
    scatter matrix W[e, d] = val_e * (row_e == d) over the chunk's 64-wide
    dest window.
  - PE (bf16) accumulates G_c^T @ W_c into the group PSUM tile -> resT.
  - GRU gate GEMMs on PE (bf16 weights, biases as per-partition ACT bias);
    elementwise on DVE/ACT in bf16.
  - PE transpose back to node-major + LayerNorm (bn_stats) + DMA out (f32).
"""

import math
import sys

import numpy as np

sys.path.insert(0, "/opt/trn_rl_repo")

import ml_dtypes  # noqa: E402

import concourse.bass as bass  # noqa: E402, F401
import concourse.tile as tile  # noqa: E402
from concourse import bacc, mybir  # noqa: E402
from concourse.bass_utils import run_bass_kernel_spmd  # noqa: E402

P = 128
GRP = 512  # dest group width (one PSUM bank of f32)
SW = 256  # GRU/LN supertile width (h state tiles)
WW = 64  # scatter window width per chunk
NCORES = 8
LN_EPS = 1e-5
GW = 1  # groups per dma_gather window
NQUEUES = 1
SEG_BUFS = 2
GATES_BUFS = 2
GATESB_BUFS = 2
LNPP_BUFS = 2
GPOOL_BUFS = 3
WPOOL_BUFS = 12
GRU_BUFS = 3
STREAM_BUFS = 2
LNP_BUFS = 8
GRU_DE_POOL = True
F32 = mybir.dt.float32
BF16 = mybir.dt.bfloat16
I16 = mybir.dt.int16
AF = mybir.ActivationFunctionType
ALU = mybir.AluOpType
BF = ml_dtypes.bfloat16


def _ceil_to(a, m):
    return (a + m - 1) // m * m


def preprocess(x, vals, rows, cols, w_x, b_x, w_h, b_h, ln_g, ln_b):
    """Host-side sharding/packing. Returns (in_maps, meta)."""
    N, D = x.shape
    assert D == P
    K, E = rows.shape
    NPAD = _ceil_to(N, NCORES * P)
    RPC = NPAD // NCORES  # rows per core
    TPC = RPC // P  # 128-tiles per core
    NG = math.ceil(RPC / GRP)  # dest groups per core
    gwid = [min(GRP, RPC - g * GRP) for g in range(NG)]  # group widths
    NST = math.ceil(RPC / SW)  # supertiles per core (for GRU/LN)
    stw = [min(SW, RPC - st * SW) for st in range(NST)]
    HALF = NPAD // 2
    assert HALF <= 32767, "dma_gather int16 index limit"

    xpad = np.zeros((NPAD, D), np.float32)
    xpad[:N] = np.asarray(x, np.float32)
    x_lo = np.ascontiguousarray(xpad[:HALF].astype(BF))
    x_hi = np.ascontiguousarray(xpad[HALF:].astype(BF))

    rows = np.asarray(rows)
    cols = np.asarray(cols)
    vals = np.asarray(vals, np.float32)

    # step j uses adjacency a = K-1-j
    # nch[j][g][s]: chunk count (shared across cores)
    # offs[j][g][s]: window offset per chunk (shared schedule)
    # per-core padded streams per (j, g, s): cl (int16), rl (window-rel f32),
    # v (f32), each nch*128 long
    nch = []
    offs = []
    core_dat = [[] for _ in range(NCORES)]  # [d][j][g][s] -> (cl, rl, v)
    for j in range(K):
        a = K - 1 - j
        r = rows[a].astype(np.int64)
        c = cols[a].astype(np.int64)
        v = vals[a]
        core = r // RPC
        rl = r % RPC
        g_arr = rl // GRP
        s_arr = c // HALF
        nch_j = []
        offs_j = []
        for d in range(NCORES):
            core_dat[d].append([])
        for g in range(NG):
            gw = gwid[g]
            ww = min(WW, gw)
            nch_g = []
            offs_g = []
            for d in range(NCORES):
                core_dat[d][j].append([])
            for s in range(2):
                # gather per-core sorted local dests + payload
                dloc_c, cl_c, v_c = [], [], []
                for d in range(NCORES):
                    m = (core == d) & (g_arr == g) & (s_arr == s)
                    dl = rl[m] - g * GRP
                    order = np.argsort(dl, kind="stable")
                    dloc_c.append(dl[order])
                    cl_c.append((c[m] % HALF)[order].astype(np.int16))
                    v_c.append(v[m][order])
                # shared schedule: min chunk-start over cores, monotone;
                # iterate greedy re-chunk until all cores fit (short chunks
                # absorb cross-core drift)
                sched = []
                n0 = max((len(dl) + P - 1) // P for dl in dloc_c)
                for k in range(n0):
                    starts = [dl[k * P] for dl in dloc_c if k * P < len(dl)]
                    off = int(max(0, min(min(starts), gw - ww)))
                    if sched:
                        off = max(off, sched[-1])
                    sched.append(off)

                def rechunk(dl, sched):
                    """Greedy chunk sorted dests against schedule.
                    Returns list of slot-index arrays per chunk, or None."""
                    out = []
                    i, n, k = 0, len(dl), 0
                    while i < n:
                        if k >= len(sched) or dl[i] < sched[k]:
                            return None
                        cnt = 0
                        while i < n and cnt < P and dl[i] < sched[k] + ww:
                            i += 1
                            cnt += 1
                        out.append((i - cnt, i))
                        k += 1
                    return out

                for _ in range(6):
                    res = [rechunk(dl, sched) for dl in dloc_c]
                    if all(r is not None for r in res):
                        break
                    # rebuild schedule from scratch per failing structure:
                    # extend with min of next unplaced dest
                    sched2 = []
                    ptrs = [0] * NCORES
                    k = 0
                    while any(ptrs[d] < len(dloc_c[d]) for d in range(NCORES)):
                        starts = [
                            dloc_c[d][ptrs[d]]
                            for d in range(NCORES)
                            if ptrs[d] < len(dloc_c[d])
                        ]
                        off = int(max(0, min(min(starts), gw - ww)))
                        if sched2:
                            off = max(off, sched2[-1])
                        sched2.append(off)
                        for d in range(NCORES):
                            dl = dloc_c[d]
                            i, cnt = ptrs[d], 0
                            while i < len(dl) and cnt < P and dl[i] < off + ww:
                                i += 1
                                cnt += 1
                            ptrs[d] = i
                        k += 1
                        assert k < 200, "schedule runaway"
                    sched = sched2
                else:
                    raise AssertionError(
                        f"window schedule infeasible j={j} g={g} s={s}"
                    )
                n_ch = max(len(r) for r in res)
                sched = sched[:n_ch] + [sched[-1]] * (n_ch - len(sched))
                # pad per-core streams to n_ch*128 (chunk-major slots)
                slots = n_ch * P
                for d in range(NCORES):
                    cl_p = np.zeros(slots, np.int16)
                    rl_p = np.zeros(slots, np.float32)
                    v_p = np.zeros(slots, np.float32)
                    for k, (lo, hi) in enumerate(res[d]):
                        cnt = hi - lo
                        sl = k * P
                        cl_p[sl : sl + cnt] = cl_c[d][lo:hi]
                        rl_p[sl : sl + cnt] = (
                            dloc_c[d][lo:hi] - sched[k]
                        ).astype(np.float32)
                        v_p[sl : sl + cnt] = v_c[d][lo:hi]
                    core_dat[d][j][g].append((cl_p, rl_p, v_p))
                nch_g.append(n_ch)
                offs_g.append(sched)
            nch_j.append(nch_g)
            offs_j.append(offs_g)
        nch.append(nch_j)
        offs.append(offs_j)

    NCH = [
        sum(nch[j][g][s] for g in range(NG) for s in range(2)) for j in range(K)
    ]
    NIDXC = [
        [sum(nch[j][g][s] for g in range(NG)) * 8 for s in range(2)]
        for j in range(K)
    ]
    windows = [(i * GW, min((i + 1) * GW, NG)) for i in range(math.ceil(NG / GW))]
    # last step: split the final groups into single-group windows so the
    # post-last-gather serial tail (W+matmul+GRU+LN) is as short as possible
    head = max(0, NG - 3)
    nh = math.ceil(head / GW)
    windows_last = [(i * GW, min((i + 1) * GW, head)) for i in range(nh)] + [
        (g, g + 1) for g in range(head, NG)
    ]

    w_x = np.asarray(w_x, np.float32)
    w_h = np.asarray(w_h, np.float32)
    b_x = np.asarray(b_x, np.float32)
    b_h = np.asarray(b_h, np.float32)
    wxT = np.ascontiguousarray(w_x.T.astype(BF))  # [128, 384]
    whT = np.ascontiguousarray(w_h.T.astype(BF))
    bias4 = np.stack(
        [
            b_x[0:P] + b_h[0:P],  # r
            b_x[P : 2 * P] + b_h[P : 2 * P],  # i
            b_x[2 * P : 3 * P],  # xn
            b_h[2 * P : 3 * P],  # hn
        ],
        axis=1,
    ).astype(np.float32)
    ln_g = np.asarray(ln_g, np.float32)
    ln_b = np.asarray(ln_b, np.float32)
    lng = np.ascontiguousarray(np.broadcast_to(ln_g[None, :], (P, P)))
    lnb = np.ascontiguousarray(np.broadcast_to(ln_b[None, :], (P, P)))
    iota = np.ascontiguousarray(
        np.broadcast_to(np.arange(WW, dtype=np.float32)[None, :], (P, WW)).astype(BF)
    )
    ident = np.eye(P, dtype=np.float32).astype(BF)

    in_maps = []
    for d in range(NCORES):
        m = dict(
            xlo=x_lo,
            xhi=x_hi,
            wxT=wxT,
            whT=whT,
            bias4=bias4,
            lng=lng,
            lnb=lnb,
            iota=iota,
            ident=ident,
        )
        for j in range(K):
            # rowf/valf column order: g-major, then s, then chunk
            rl_flat = np.concatenate(
                [core_dat[d][j][g][s][1] for g in range(NG) for s in range(2)]
            )
            v_flat = np.concatenate(
                [core_dat[d][j][g][s][2] for g in range(NG) for s in range(2)]
            )
            m[f"rowf{j}"] = np.ascontiguousarray(rl_flat.reshape(NCH[j], P).T)
            m[f"valf{j}"] = np.ascontiguousarray(v_flat.reshape(NCH[j], P).T)
            for s in range(2):
                cl_flat = np.concatenate(
                    [core_dat[d][j][g][s][0] for g in range(NG)]
                )
                arr = np.zeros((P, max(NIDXC[j][s], 8)), np.int16)
                if cl_flat.size:
                    # wrapped in 16 partitions, replicated across the 8
                    # GPSIMD cores (partition groups of 16)
                    wrap = cl_flat.reshape(-1, 16).T
                    arr[:, : cl_flat.size // 16] = np.tile(wrap, (8, 1))
                m[f"idx{j}{s}"] = arr
        in_maps.append(m)

    meta = dict(
        N=N,
        D=D,
        K=K,
        NPAD=NPAD,
        RPC=RPC,
        TPC=TPC,
        NG=NG,
        gwid=gwid,
        NST=NST,
        stw=stw,
        HALF=HALF,
        nch=nch,
        offs=offs,
        NCH=NCH,
        NIDXC=NIDXC,
        windows=windows,
        windows_last=windows_last,
        skip_g=bool(np.allclose(ln_g, 1.0)),
        skip_b=bool(np.allclose(ln_b, 0.0)),
    )
    return in_maps, meta


def build_program(meta):
    """Build the single-core SPMD Bass program."""
    K, NG, HALF, D = meta["K"], meta["NG"], meta["HALF"], meta["D"]
    RPC, gwid, stw = meta["RPC"], meta["gwid"], meta["stw"]
    nch, offs = meta["nch"], meta["offs"]
    NCH, NIDXC, windows = meta["NCH"], meta["NIDXC"], meta["windows"]
    windows_last = meta.get("windows_last", windows)
    all_windows = list(windows) + list(windows_last)

    def win_cw(j, w0, w1, s):
        return sum(nch[j][g][s] for g in range(w0, w1))

    maxidx = (
        max(
            win_cw(j, w0, w1, s)
            for j in range(K)
            for (w0, w1) in all_windows
            for s in range(2)
        )
        * P
    )
    scratch = max(16384, _ceil_to((maxidx + 512) * 16, 4096))
    nc = bacc.Bacc(
        "TRN2",
        target_bir_lowering=False,
        debug=False,
        dynamic_dma_scratch_size=scratch,
        num_swdge_queues=NQUEUES,
    )

    xsrc = [
        nc.dram_tensor("xlo", [HALF, D], BF16, kind="ExternalInput").ap(),
        nc.dram_tensor("xhi", [HALF, D], BF16, kind="ExternalInput").ap(),
    ]
    wxT_d = nc.dram_tensor("wxT", [P, 3 * P], BF16, kind="ExternalInput").ap()
    whT_d = nc.dram_tensor("whT", [P, 3 * P], BF16, kind="ExternalInput").ap()
    bias_d = nc.dram_tensor("bias4", [P, 4], F32, kind="ExternalInput").ap()
    lng_d = nc.dram_tensor("lng", [P, P], F32, kind="ExternalInput").ap()
    lnb_d = nc.dram_tensor("lnb", [P, P], F32, kind="ExternalInput").ap()
    iota_d = nc.dram_tensor("iota", [P, WW], BF16, kind="ExternalInput").ap()
    ident_d = nc.dram_tensor("ident", [P, P], BF16, kind="ExternalInput").ap()
    rowf_d = [
        nc.dram_tensor(f"rowf{j}", [P, NCH[j]], F32, kind="ExternalInput").ap()
        for j in range(K)
    ]
    valf_d = [
        nc.dram_tensor(f"valf{j}", [P, NCH[j]], F32, kind="ExternalInput").ap()
        for j in range(K)
    ]
    idx_d = [
        [
            nc.dram_tensor(
                f"idx{j}{s}", [P, max(NIDXC[j][s], 8)], I16, kind="ExternalInput"
            ).ap()
            for s in range(2)
        ]
        for j in range(K)
    ]
    out_d = nc.dram_tensor("out", [RPC, D], F32, kind="ExternalOutput").ap()

    nchmax = max(NCH)
    nidxmax = max(max(NIDXC[j]) for j in range(K))
    cwmax = max(
        win_cw(j, w0, w1, s)
        for j in range(K)
        for (w0, w1) in all_windows
        for s in range(2)
    )

    with tile.TileContext(nc) as tc:
        with (
            tc.tile_pool(name="const", bufs=1) as const,
            tc.tile_pool(name="stream", bufs=STREAM_BUFS) as stream,
            tc.tile_pool(name="gpool", bufs=GPOOL_BUFS) as gpool,
            tc.tile_pool(name="wpool", bufs=WPOOL_BUFS) as wpool,
            tc.tile_pool(name="gru", bufs=GRU_BUFS) as gru,
            tc.tile_pool(name="lnp", bufs=LNP_BUFS) as lnp,
            tc.tile_pool(name="psum", bufs=2, space="PSUM") as psum,
        ):
            # constants
            iota_t = const.tile([P, WW], BF16)
            nc.sync.dma_start(out=iota_t[:], in_=iota_d[:])
            ident_t = const.tile([P, P], BF16)
            nc.sync.dma_start(out=ident_t[:], in_=ident_d[:])
            wxT_t = const.tile([P, 3 * P], BF16)
            nc.sync.dma_start(out=wxT_t[:], in_=wxT_d[:])
            whT_t = const.tile([P, 3 * P], BF16)
            nc.sync.dma_start(out=whT_t[:], in_=whT_d[:])
            bias_t = const.tile([P, 4], F32)
            nc.sync.dma_start(out=bias_t[:], in_=bias_d[:])
            lng_t = const.tile([P, P], F32)
            nc.sync.dma_start(out=lng_t[:], in_=lng_d[:])
            lnb_t = const.tile([P, P], F32)
            nc.sync.dma_start(out=lnb_t[:], in_=lnb_d[:])
            zcol_t = const.tile([P, 1], F32)
            nc.vector.memset(zcol_t[:], 0.0)
            zres_t = const.tile([P, GRP], BF16)
            nc.vector.memset(zres_t[:], 0.0)
            eps_t = const.tile([P, 1], F32)
            nc.vector.memset(eps_t[:], LN_EPS)

            h_t = [
                const.tile([P, SW], BF16, tag=f"h{t}", name=f"h{t}")
                for t in range(meta["NST"])
            ]

            def ln_tile(st, off):
                """LayerNorm + store for the 128-row tile at h_t[st][:, off:]."""
                tt = (st * SW + off) // P
                hp = psum.tile([P, P], BF16, tag="lnp", space="PSUM", name="hp",
                               bufs=LNPP_BUFS)
                nc.tensor.transpose(hp[:], h_t[st][:, off : off + P], ident_t[:])
                stats = lnp.tile([P, 6], F32, tag="stats", name="stats")
                nc.vector.bn_stats(out=stats[:], in_=hp[:])
                mv = lnp.tile([P, 2], F32, tag="mv", name="mv")
                nc.vector.bn_aggr(out=mv[:], in_=stats[:])
                sd = lnp.tile([P, 1], F32, tag="sd", name="sd")
                nc.scalar.activation(
                    out=sd[:], in_=mv[:, 1:2], func=AF.Sqrt, bias=eps_t[:, 0:1]
                )
                rstd = lnp.tile([P, 1], F32, tag="rstd", name="rstd")
                nc.vector.reciprocal(out=rstd[:], in_=sd[:])
                nmr = lnp.tile([P, 1], F32, tag="nmr", name="nmr")
                nc.vector.tensor_scalar(
                    out=nmr[:],
                    in0=mv[:, 0:1],
                    scalar1=rstd[:, 0:1],
                    scalar2=-1.0,
                    op0=ALU.mult,
                    op1=ALU.mult,
                )
                o_t = lnp.tile([P, P], F32, tag="o", name="o")
                nc.scalar.activation(
                    out=o_t[:],
                    in_=hp[:],
                    func=AF.Identity,
                    bias=nmr[:, 0:1],
                    scale=rstd[:, 0:1],
                )
                if not meta["skip_g"]:
                    o2 = lnp.tile([P, P], F32, tag="o2", name="o2")
                    nc.vector.tensor_tensor(
                        out=o2[:], in0=o_t[:], in1=lng_t[:], op=ALU.mult
                    )
                    o_t = o2
                if not meta["skip_b"]:
                    o3 = lnp.tile([P, P], F32, tag="o3", name="o3")
                    nc.vector.tensor_tensor(
                        out=o3[:], in0=o_t[:], in1=lnb_t[:], op=ALU.add
                    )
                    o_t = o3
                nc.sync.dma_start(out=out_d[tt * P : (tt + 1) * P, :], in_=o_t[:])

            def gru_supertile(j, st, rsrc, roff):
                """GRU cell for supertile st; res slice rsrc[:, roff:roff+width]."""
                width = stw[st]
                gpA = psum.tile(
                    [P, 2, SW], F32, tag="gatesA", space="PSUM",
                    bufs=GATES_BUFS, name="gpA",
                )
                gpB = psum.tile(
                    [P, 2, SW], F32, tag="gatesB", space="PSUM",
                    bufs=GATESB_BUFS, name="gpB",
                )
                lastA = 1 if j == 0 else 3
                mmA = 0
                mmB = 0
                nmmB = 1 if j == 0 else 2

                def mmx(gi, wt, wcol, rhs):
                    nonlocal mmA, mmB
                    if gi < 2:
                        out = gpA[:, gi, :width]
                        st_, sp_ = mmA == 0, mmA == lastA
                        mmA += 1
                    else:
                        out = gpB[:, gi - 2, :width]
                        st_, sp_ = mmB == 0, mmB == nmmB - 1
                        mmB += 1
                    nc.tensor.matmul(
                        out,
                        lhsT=wt[:, wcol : wcol + P],
                        rhs=rhs,
                        start=st_,
                        stop=sp_,
                    )

                rcur = rsrc[:, roff : roff + width]
                mmx(0, wxT_t, 0, rcur)
                mmx(1, wxT_t, P, rcur)
                mmx(2, wxT_t, 2 * P, rcur)
                if j > 0:
                    hcur = h_t[st][:, :width]
                    mmx(0, whT_t, 0, hcur)
                    mmx(1, whT_t, P, hcur)
                    mmx(3, whT_t, 2 * P, hcur)
                r_t = gru.tile([P, SW], BF16, tag="r")
                nc.scalar.activation(
                    out=r_t[:, :width],
                    in_=gpA[:, 0, :width],
                    func=AF.Sigmoid,
                    bias=bias_t[:, 0:1],
                )
                i_t = gru.tile([P, SW], BF16, tag="i")
                nc.scalar.activation(
                    out=i_t[:, :width],
                    in_=gpA[:, 1, :width],
                    func=AF.Sigmoid,
                    bias=bias_t[:, 1:2],
                )
                t1 = gru.tile([P, SW], BF16, tag="t1")
                if j > 0:
                    nc.vector.scalar_tensor_tensor(
                        out=t1[:, :width],
                        in0=gpB[:, 1, :width],
                        scalar=bias_t[:, 3:4],
                        in1=r_t[:, :width],
                        op0=ALU.add,
                        op1=ALU.mult,
                    )
                else:
                    nc.vector.tensor_scalar(
                        out=t1[:, :width],
                        in0=r_t[:, :width],
                        scalar1=bias_t[:, 3:4],
                        scalar2=None,
                        op0=ALU.mult,
                    )
                t2 = gru.tile([P, SW], BF16, tag="t2")
                nc.vector.tensor_tensor(
                    out=t2[:, :width],
                    in0=t1[:, :width],
                    in1=gpB[:, 0, :width],
                    op=ALU.add,
                )
                nn = gru.tile([P, SW], BF16, tag="nn")
                nc.scalar.activation(
                    out=nn[:, :width],
                    in_=t2[:, :width],
                    func=AF.Tanh,
                    bias=bias_t[:, 2:3],
                )
                if j > 0:
                    deng = nc.gpsimd if GRU_DE_POOL else nc.vector
                    d_t = gru.tile([P, SW], BF16, tag="d")
                    deng.tensor_tensor(
                        out=d_t[:, :width],
                        in0=h_t[st][:, :width],
                        in1=nn[:, :width],
                        op=ALU.subtract,
                    )
                    e_t = gru.tile([P, SW], BF16, tag="e")
                    deng.tensor_tensor(
                        out=e_t[:, :width],
                        in0=i_t[:, :width],
                        in1=d_t[:, :width],
                        op=ALU.mult,
                    )
                    nc.vector.tensor_tensor(
                        out=h_t[st][:, :width],
                        in0=nn[:, :width],
                        in1=e_t[:, :width],
                        op=ALU.add,
                    )
                else:
                    om = gru.tile([P, SW], BF16, tag="om")
                    nc.vector.tensor_scalar(
                        out=om[:, :width],
                        in0=i_t[:, :width],
                        scalar1=1.0,
                        scalar2=-1.0,
                        op0=ALU.subtract,
                        op1=ALU.mult,
                    )
                    nc.vector.tensor_tensor(
                        out=h_t[st][:, :width],
                        in0=nn[:, :width],
                        in1=om[:, :width],
                        op=ALU.mult,
                    )
                if j == K - 1:
                    for off in range(0, width, P):
                        ln_tile(st, off)

            for j in range(K):
                rowf_t = stream.tile([P, nchmax], F32, tag="rowf")
                nc.sync.dma_start(out=rowf_t[:, : NCH[j]], in_=rowf_d[j][:])
                valf_t = stream.tile([P, nchmax], F32, tag="valf")
                nc.sync.dma_start(out=valf_t[:, : NCH[j]], in_=valf_d[j][:])
                idx_t = []
                for s in range(2):
                    it = stream.tile([P, nidxmax], I16, tag=f"idx{s}")
                    if NIDXC[j][s]:
                        nc.sync.dma_start(
                            out=it[:, : NIDXC[j][s]], in_=idx_d[j][s][:]
                        )
                    idx_t.append(it)

                ch_col = 0  # chunk column into rowf/valf (g-major, s, k)
                idx_chunk_off = [0, 0]  # chunk offset within (j, s) idx stream
                gctr = j
                for (w0, w1) in (windows_last if j == K - 1 else windows):
                    cws = [win_cw(j, w0, w1, s) for s in range(2)]
                    g_t = []
                    for s in range(2):
                        if cws[s] == 0:
                            g_t.append(None)
                            continue
                        g = gpool.tile([P, cwmax, P], BF16, tag=f"g{s}")
                        nc.gpsimd.dma_gather(
                            g[:, : cws[s], :],
                            xsrc[s][:],
                            idx_t[s][
                                :,
                                idx_chunk_off[s] * 8 : (idx_chunk_off[s] + cws[s])
                                * 8,
                            ],
                            num_idxs=cws[s] * P,
                            num_idxs_reg=cws[s] * P,
                            elem_size=D,
                            single_packet=False,
                            queue_num=gctr % NQUEUES,
                        )
                        gctr += 1
                        g_t.append(g)
                    idx_chunk_off[0] += cws[0]
                    idx_chunk_off[1] += cws[1]
                    gloc = [0, 0]  # chunk cursor within this window per s
                    for g in range(w0, w1):
                        gw = gwid[g]
                        ntot = nch[j][g][0] + nch[j][g][1]
                        segp = psum.tile(
                            [P, GRP], F32, tag="seg", space="PSUM",
                            bufs=SEG_BUFS,
                        )
                        # zero the whole group accumulator (start=True marks
                        # the full 2KB region; the full-width write clears it
                        # uniformly so overlapping chunk windows can accumulate)
                        nc.tensor.matmul(
                            segp[:, :gw],
                            lhsT=zres_t[:, :P],
                            rhs=zres_t[:, :gw],
                            start=True,
                            stop=False,
                            skip_group_check=True,
                        )
                        ci_done = 0
                        for s in range(2):
                            for k in range(nch[j][g][s]):
                                off = offs[j][g][s][k]
                                wwk = min(WW, gw)
                                w_tile = wpool.tile([P, WW], BF16, tag="w")
                                nc.vector.tensor_scalar(
                                    out=w_tile[:, :wwk],
                                    in0=iota_t[:, :wwk],
                                    scalar1=rowf_t[:, ch_col : ch_col + 1],
                                    scalar2=valf_t[:, ch_col : ch_col + 1],
                                    op0=ALU.is_equal,
                                    op1=ALU.mult,
                                )
                                nc.tensor.matmul(
                                    segp[:, off : off + wwk],
                                    lhsT=g_t[s][:, gloc[s] + k, :],
                                    rhs=w_tile[:, :wwk],
                                    start=False,
                                    stop=(ci_done == ntot - 1),
                                    skip_group_check=True,
                                )
                                ch_col += 1
                                ci_done += 1
                            gloc[s] += nch[j][g][s]
                        # relu PSUM -> SBUF for the whole group
                        resT = gru.tile([P, GRP], BF16, tag="resT")
                        nc.scalar.activation(
                            out=resT[:, :gw],
                            in_=segp[:, :gw],
                            func=AF.Relu,
                            bias=zcol_t[:, 0:1],
                        )
                        # GRU on the supertiles of this group
                        for u in range(math.ceil(gw / SW)):
                            st = g * (GRP // SW) + u
                            gru_supertile(j, st, resT, u * SW)

    nc.compile()
    return nc


def prepare(inputs):
    in_maps, meta = preprocess(
        inputs["x"],
        inputs["vals"],
        inputs["rows"],
        inputs["cols"],
        inputs["w_x"],
        inputs["b_x"],
        inputs["w_h"],
        inputs["b_h"],
        inputs["ln_g"],
        inputs["ln_b"],
    )
    nc = build_program(meta)
    return nc, in_maps, meta


def kernel(**inputs) -> np.ndarray:
    nc, in_maps, meta = prepare(inputs)
    res = run_bass_kernel_spmd(nc, in_maps, core_ids=list(range(NCORES)))
    outs = [res.results[d]["out"] for d in range(NCORES)]
    full = np.concatenate(outs, axis=0)[: meta["N"]]
    return full.astype(np.float32)
